# revision 27
# baseline (speedup 1.0000x reference)
"""Bass/Trainium2 kernel for nn_BayesMultiheadAttention (B=4,T=2048,D=1024,H=8).

Sharding: tensor-parallel over heads. Core c computes head c (QKV proj +
causal attention) for all 4 batches; a per-batch fp16 AllToAll
redistributes per-head outputs into per-token-slice outputs (consumed two
batches later, so the collective is never on the critical path); each
core then does the multiplicative reduce over heads and its slice of
out_proj.

x and the QKV weights ship as fp16 (halves load DMA; ~same precision as
the PE's internal f32r 11-bit rounding). All matmuls run at full PE rate:
fp16 for QKV, f32r elsewhere. V is projected at N=512 then flipped with
PE transposes. Softmax denominators: e-tiles are pre-summed in pairs
(DVE/Pool) so PE runs half as many ones-matmuls; softmax skips
max-subtraction (scores are O(5), exp cannot overflow). Dropout masks and
the 1/sqrt(HD) scale are folded into per-(core,batch) weight copies on
the host, which also pre-transposes to per-partition-contiguous layouts
so every steady-state load is one large clean DMA.

Queues: PE matmuls only; ACT exp (+ startup loads); DVE masks/recip/
normalize + es-adds; Pool collectives, a2a staging, tail gather+product
chain, es-adds; SP x/w loads and y stores. PSUM evictions use nc.any so
the tile scheduler balances engines. build_program(reps=k) chains the
whole pipeline k times in one NEFF for launch-overhead-free timing.
"""
import numpy as np

import concourse.bacc as bacc
import concourse.mybir as mybir
import concourse.tile as tile
from concourse.bass_utils import run_bass_kernel_spmd

B, T, D, H = 4, 2048, 1024, 8
HD = 128          # head dim
P = 128           # partitions
NC = 8            # cores
TQ = 512          # qt chunk width
NKD = D // P      # 8 contraction tiles
NTT = T // P      # 16 token tiles per batch
NQC = T // TQ     # 4 qt chunks per batch
TS = T // NC      # 256: per-core token slice of one batch
TOK_SLICE = B * TS  # 1024 tokens per core in the tail

dt = mybir.dt
F32 = dt.float32
F32R = dt.float32r
F16 = dt.float16

_PROGRAM = {}


def build_program(mode=None, reps=1):
    """reps>1 repeats the whole pipeline inside one NEFF (for timing)."""
    global _PROGRAM
    key = f"v2-{reps}"
    if key in _PROGRAM:
        return _PROGRAM[key]
    nc = bacc.Bacc("TRN2", target_bir_lowering=False, debug=False,
                   num_devices=NC)

    xT_d = nc.dram_tensor("xT", [B, D, T], F16, kind="ExternalInput")
    wq_d = nc.dram_tensor("wq", [B, P, NKD * HD], F16, kind="ExternalInput")
    wk_d = nc.dram_tensor("wk", [B, P, NKD * HD], F16, kind="ExternalInput")
    wv_d = nc.dram_tensor("wv", [B, P, NKD * HD], F16, kind="ExternalInput")
    wo_d = nc.dram_tensor("wo", [HD, D], F32R, kind="ExternalInput")
    cm_d = nc.dram_tensor("cm", [4, P, TQ], F32, kind="ExternalInput")
    eye_d = nc.dram_tensor("eye", [P, P], F32R, kind="ExternalInput")
    y_d = nc.dram_tensor("y", [TOK_SLICE, D], F32, kind="ExternalOutput")

    rg = [list(range(NC))]
    Exp = mybir.ActivationFunctionType.Exp

    from contextlib import ExitStack
    with tile.TileContext(nc) as tc, ExitStack() as ctx:
        ec = ctx.enter_context
        constp = ec(tc.tile_pool(name="const", bufs=1))
        xp = ec(tc.tile_pool(name="xp", bufs=2))
        wsp = ec(tc.tile_pool(name="wsp", bufs=2))
        qkvp = ec(tc.tile_pool(name="qkv", bufs=1))
        eop = ec(tc.tile_pool(name="eo", bufs=8))
        esp = ec(tc.tile_pool(name="es", bufs=4))
        scp = ec(tc.tile_pool(name="sc", bufs=3))
        outbp = ec(tc.tile_pool(name="outb", bufs=1))
        prodp = ec(tc.tile_pool(name="prod", bufs=2))
        ysbp = ec(tc.tile_pool(name="ysb", bufs=2))
        psA = ec(tc.tile_pool(name="psA", bufs=2, space="PSUM"))
        psS = ec(tc.tile_pool(name="psS", bufs=2, space="PSUM"))
        psO = ec(tc.tile_pool(name="psO", bufs=2, space="PSUM"))
        psD = ec(tc.tile_pool(name="psD", bufs=2, space="PSUM"))
        dram = ec(tc.tile_pool(name="dram", bufs=1, space="DRAM"))

        a2a_in = [dram.tile([NC, P, TS], F16, name=f"a2a_in{b}",
                            tag=f"a2a_in{b}") for b in range(B)]
        a2a_out = [dram.tile([NC, P, TS], F16, name=f"a2a_out{b}",
                             tag=f"a2a_out{b}") for b in range(B)]
        hpp = ec(tc.tile_pool(name="hp", bufs=2))

        ones_st = constp.tile([P, P], F32, name="ones_st", tag="ones_st")
        nc.vector.memset(ones_st[:], 1.0)
        ones_r = constp.tile([P, P], F32R, name="ones_r", tag="ones_r")
        nc.vector.tensor_copy(ones_r[:], ones_st[:])

        eye_r = constp.tile([P, P], F32R, name="eye_r", tag="eye_r")
        nc.sync.dma_start(eye_r[:], eye_d.ap())

        cm_sb = constp.tile([P, 4 * TQ], F32, name="cm_sb", tag="cm_sb")

        wor = constp.tile([P, D], F32R, name="wor", tag="wor")

        # ---- load machinery -------------------------------------------
        staged = {}

        def emit_loads(bb, startup=False):
            """DMA batch bb's x chunks + weights; record tiles in staged."""
            b = bb % B
            st = {}
            staged[bb] = st
            x_sb = xp.tile([P, NKD * T], F16, name="x_sb", tag="x_sb")
            st["x"] = x_sb
            ws = {}
            for nm, wd in (("v", wv_d), ("q", wq_d), ("k", wk_d)):
                ws[nm] = wsp.tile([P, NKD * HD], F16, name=f"ws_{nm}",
                                  tag=f"ws_{nm}")
            st["w"] = ws
            if startup:
                # spread batch-0 loads over Pool/SP/ACT in half-chunk DMAs
                # so the kd-outer v projection starts ~2us in and is never
                # starved; wv first on ACT
                nc.scalar.dma_start(ws["v"][:], wv_d.ap()[b])
                engs = [nc.gpsimd, nc.sync, nc.scalar]
                i = 0
                HT = T // 2
                QT4 = T // 4
                for kd in range(NKD):
                    if kd == 0:
                        for h in range(4):
                            engs[i % 3].dma_start(
                                x_sb[:, h * QT4:(h + 1) * QT4],
                                xT_d.ap()[b, 0:P, h * QT4:(h + 1) * QT4])
                            i += 1
                        continue
                    for h in range(2):
                        engs[i % 3].dma_start(
                            x_sb[:, kd * T + h * HT: kd * T + (h + 1) * HT],
                            xT_d.ap()[b, kd * P:(kd + 1) * P,
                                      h * HT:(h + 1) * HT])
                        i += 1
                    if kd == 2:
                        nc.sync.dma_start(ws["q"][:], wq_d.ap()[b])
                    elif kd == 4:
                        nc.gpsimd.dma_start(ws["k"][:], wk_d.ap()[b])
                return
            # steady state: all on SP; few big DMAs (HW A/B showed the
            # consolidated transfers beat per-chunk DMAs by ~90us)
            HK = NKD // 2
            nc.sync.dma_start(ws["v"][:], wv_d.ap()[b])
            nc.sync.dma_start(
                x_sb[:, 0:HK * T],
                xT_d.ap()[b, 0:HK * P, :].rearrange("(kd p) t -> p kd t",
                                                    p=P))
            nc.sync.dma_start(ws["q"][:], wq_d.ap()[b])
            nc.sync.dma_start(
                x_sb[:, HK * T:NKD * T],
                xT_d.ap()[b, HK * P:NKD * P, :].rearrange(
                    "(kd p) t -> p kd t", p=P))
            nc.sync.dma_start(ws["k"][:], wk_d.ap()[b])

        def emit_tail(bb, final=False):
            """Consume A2A(bb): f16 head-product chain (Pool only), out_proj."""
            b = bb % B
            hp = hpp.tile([P, NC * TS], F16, name="hp", tag="hp")
            if final:
                half = NC // 2
                nc.gpsimd.dma_start(
                    hp[:, 0:half * TS],
                    a2a_out[b][0:half].rearrange("r p t -> p r t"))
                nc.scalar.dma_start(
                    hp[:, half * TS:],
                    a2a_out[b][half:NC].rearrange("r p t -> p r t"))
            else:
                nc.gpsimd.dma_start(
                    hp[:], a2a_out[b].rearrange("r p t -> p r t"))
            pr = prodp.tile([P, TS], F16, name="pr", tag="pr")
            nc.gpsimd.tensor_mul(pr[:], hp[:, 0:TS], hp[:, TS:2 * TS])
            for r in range(2, NC - 1):
                nc.gpsimd.tensor_mul(
                    pr[:], pr[:], hp[:, r * TS:(r + 1) * TS])
            prod_r = prodp.tile([P, TS], F32R, name="prod_r", tag="prodr")
            nc.gpsimd.tensor_mul(
                prod_r[:], pr[:], hp[:, (NC - 1) * TS:NC * TS])
            for tt in range(TS // P):
                ysb = ysbp.tile([P, D], F32, name="ysb", tag="ysb")
                for nn in range(D // TQ):
                    accy = psA.tile([P, TQ], F32, name="accy", tag="mmacc")
                    nc.tensor.matmul(
                        accy[:],
                        prod_r[:, tt * P:(tt + 1) * P],
                        wor[:, nn * TQ:(nn + 1) * TQ],
                        start=True, stop=True)
                    nc.vector.tensor_copy(
                        ysb[:, nn * TQ:(nn + 1) * TQ], accy[:])
                row = b * TS + tt * P
                yeng = nc.scalar if (final and tt == 1) else nc.sync
                yeng.dma_start(y_d.ap()[row:row + P, :], ysb[:])

        emit_loads(0, startup=True)
        nc.scalar.dma_start(cm_sb[:], cm_d.ap().rearrange("j p q -> p j q"))
        nc.gpsimd.dma_start(wor[:], wo_d.ap())

        NB = B * reps
        for bb in range(NB):
            b = bb % B
            st = staged.pop(bb)
            if bb + 1 < NB:
                emit_loads(bb + 1)
            xr = st["x"]
            wvr = st["w"]["v"]
            wqr = st["w"]["q"]
            wkr = st["w"]["k"]

            # ---- V projection, kd-outer (4 chunk accumulators) --------
            vT = qkvp.tile([P, T], F32R, name="vT", tag="vT")
            vaccs = [(psS if i < 2 else psO).tile(
                [P, TQ], F32, name=f"vacc{i}",
                tag="accs" if i < 2 else "acco") for i in range(NQC)]
            for kd in range(NKD):
                for qc in range(NQC):
                    nc.tensor.matmul(
                        vaccs[qc][:],
                        wvr[:, kd * HD:(kd + 1) * HD],
                        xr[:, kd * T + qc * TQ: kd * T + (qc + 1) * TQ],
                        start=(kd == 0), stop=(kd == NKD - 1))
            for qc in range(NQC):
                nc.any.tensor_copy(
                    vT[:, qc * TQ:(qc + 1) * TQ], vaccs[qc][:])

            # flip V to (token partitions, hd free) via PE transposes
            v_sb = qkvp.tile([P, NTT * HD], F32R, name="vS", tag="vS")
            for tt in range(NTT):
                vtp = psA.tile([P, P], F32R, name="vtp", tag="mmacc")
                nc.tensor.transpose(
                    vtp[:], vT[:, tt * P:(tt + 1) * P], eye_r[:])
                nc.any.tensor_copy(
                    v_sb[:, tt * HD:(tt + 1) * HD], vtp[:])

            # ---- Q,K projections, kd-outer (8 chunk accumulators) -----
            qT = qkvp.tile([P, T], F32R, name="qT", tag="qT")
            kT = qkvp.tile([P, T], F32R, name="kT", tag="kT")
            qaccs = [(psS if i < 2 else psO).tile(
                [P, TQ], F32, name=f"qacc{i}",
                tag="accs" if i < 2 else "acco") for i in range(NQC)]
            kaccs = [(psD if i < 2 else psA).tile(
                [P, TQ], F32, name=f"kacc{i}",
                tag="denb" if i < 2 else "mmacc") for i in range(NQC)]
            for kd in range(NKD):
                for qc in range(NQC):
                    nc.tensor.matmul(
                        qaccs[qc][:],
                        wqr[:, kd * HD:(kd + 1) * HD],
                        xr[:, kd * T + qc * TQ: kd * T + (qc + 1) * TQ],
                        start=(kd == 0), stop=(kd == NKD - 1))
                for qc in range(NQC):
                    nc.tensor.matmul(
                        kaccs[qc][:],
                        wkr[:, kd * HD:(kd + 1) * HD],
                        xr[:, kd * T + qc * TQ: kd * T + (qc + 1) * TQ],
                        start=(kd == 0), stop=(kd == NKD - 1))
            for qc in range(NQC):
                nc.any.tensor_copy(
                    qT[:, qc * TQ:(qc + 1) * TQ], qaccs[qc][:])
                nc.any.tensor_copy(
                    kT[:, qc * TQ:(qc + 1) * TQ], kaccs[qc][:])

            # ---- causal attention, scoresT layout ----------------------
            out_b = outbp.tile([P, T], F16, name="out_b", tag="out_b")
            for qc in range(NQC):
                nkt = 4 * (qc + 1)
                acco = psO.tile([P, TQ], F32, name="acco", tag="acco")
                denb = psD.tile([P, TQ], F32, name="denb", tag="denb")
                prev_e = None
                for kt in range(nkt):
                    accs = psS.tile([P, TQ], F32, name="accs", tag="accs")
                    nc.tensor.matmul(
                        accs[:],
                        kT[:, kt * P:(kt + 1) * P],
                        qT[:, qc * TQ:(qc + 1) * TQ],
                        start=True, stop=True)
                    e = eop.tile([P, TQ], F32R, name="e", tag="e")
                    nc.scalar.activation(e[:], accs[:], Exp)
                    j = kt - 4 * qc
                    if j >= 0:  # diagonal-crossing tile: zero invalid
                        nc.vector.tensor_mul(
                            e[:], e[:], cm_sb[:, j * TQ:(j + 1) * TQ])
                    nc.tensor.matmul(
                        acco[:],
                        v_sb[:, kt * HD:(kt + 1) * HD],
                        e[:],
                        start=(kt == 0), stop=(kt == nkt - 1))
                    # softmax denominator: pre-sum e pairs on DVE so PE
                    # only runs half as many ones-matmuls
                    if kt % 2 == 0:
                        prev_e = e
                    else:
                        es = esp.tile([P, TQ], F32R, name="es", tag="es")
                        nc.vector.tensor_add(es[:], prev_e[:], e[:])
                        nc.tensor.matmul(
                            denb[:], ones_r[:], es[:],
                            start=(kt == 1), stop=(kt == nkt - 1))
                recb = scp.tile([P, TQ], F32, name="recb", tag="recb")
                nc.vector.reciprocal_approx_fast(recb[:], denb[:])
                nc.vector.tensor_mul(
                    out_b[:, qc * TQ:(qc + 1) * TQ], acco[:], recb[:])
                # ship this qc's two token slices to the collective buffer
                for j in (2 * qc, 2 * qc + 1):
                    nc.gpsimd.dma_start(a2a_in[b][j],
                                        out_b[:, j * TS:(j + 1) * TS])

                if qc == 1 and bb > 1:
                    emit_tail(bb - 2)

            # ---- ship normalized head-output (f16) ---------------------
            nc.gpsimd.collective_compute(
                "AllToAll", mybir.AluOpType.bypass,
                replica_groups=rg,
                ins=[a2a_in[b].opt()], outs=[a2a_out[b].opt()])

        emit_tail(NB - 2)
        emit_tail(NB - 1, final=True)

    nc.compile()
    _PROGRAM[key] = nc
    return nc


def make_in_maps(x, Wq, Wk, Wv, Wout, q_mask, k_mask, v_mask):
    x = np.ascontiguousarray(np.asarray(x, np.float32))
    xT = np.ascontiguousarray(x.transpose(0, 2, 1).astype(np.float16))
    wo = np.ascontiguousarray(np.asarray(Wout, np.float32).T)  # (HD, D)

    cm = np.zeros((4, P, TQ), np.float32)
    for j in range(4):
        for i in range(P):
            cm[j, i, j * P + i:] = 1.0
    eye = np.eye(P, dtype=np.float32)

    s = np.float32(1.0 / np.sqrt(HD))
    q_mask = np.asarray(q_mask, np.float32)
    k_mask = np.asarray(k_mask, np.float32)
    v_mask = np.asarray(v_mask, np.float32)
    Wq = np.asarray(Wq, np.float32)
    Wk = np.asarray(Wk, np.float32)
    Wv = np.asarray(Wv, np.float32)

    in_maps = []
    for c in range(NC):
        def pack(W, m, scale):
            out = np.empty((B, P, NKD * HD), np.float16)
            Wh = W[c * HD:(c + 1) * HD, :]                  # (HD, D)
            for b in range(B):
                Wp = (Wh * (m[b, c, 0, :, None] * scale)).T  # (D, HD)
                out[b] = Wp.reshape(NKD, P, HD).transpose(1, 0, 2).reshape(
                    P, NKD * HD)
            return out
        in_maps.append({
            "xT": xT,
            "wq": pack(Wq, q_mask, s),
            "wk": pack(Wk, k_mask, np.float32(1.0)),
            "wv": pack(Wv, v_mask, np.float32(1.0)),
            "wo": wo,
            "cm": cm,
            "eye": eye,
        })
    return in_maps


def kernel(x, Wq, Wk, Wv, Wout, q_mask, k_mask, v_mask, mask=None):
    nc = build_program()
    in_maps = make_in_maps(x, Wq, Wk, Wv, Wout, q_mask, k_mask, v_mask)
    res = run_bass_kernel_spmd(nc, in_maps, core_ids=list(range(NC))).results
    # core c's y rows are ordered (b, local-token); its tokens are
    # [c*TS, (c+1)*TS) of every batch
    out = np.empty((B, T, D), np.float32)
    for c in range(NC):
        yc = res[c]["y"].reshape(B, TS, D)
        out[:, c * TS:(c + 1) * TS, :] = yc
    return out
